# revision 5
# baseline (speedup 1.0000x reference)
"""Tensor-parallel multi-head attention (RoPE) for 8 Trainium2 NeuronCores.

Sharding: heads split 8 ways (4 heads/core). Each core computes its heads'
Q/K/V projections, RoPE, scores^T, exp (mask folded into the activation
bias), context (with an appended ones-column producing softmax row-sums for
free), and a partial output projection. Host assembles: normalizes +
transposes attn weights, sums partial output projections, adds biases.

Device-side layouts (per core):
  - q/k head dims are reordered as [head, half, j] so RoPE's rotate-half is
    a partition-aligned operation across all 4 heads at once.
  - scores are computed transposed (S^T[sk, sq]) so softmax's exp takes the
    attention mask as a per-partition bias, E^T feeds the context matmul
    directly as lhsT, and no on-device transposes are needed anywhere.
"""

import os
import sys

import numpy as np

for _p in ("/opt/trn_rl_repo", "/root/.axon_site/_ro/trn_rl_repo"):
    if os.path.isdir(_p) and _p not in sys.path:
        sys.path.insert(0, _p)

import concourse.bass as bass  # noqa: E402
import concourse.mybir as mybir  # noqa: E402
from concourse import bacc  # noqa: E402
from concourse.bass_utils import run_bass_kernel_spmd  # noqa: E402
from concourse.tile import TileContext  # noqa: E402

B, HID, NH, HD = 2, 2048, 32, 64
NCORES = 8
HPC = NH // NCORES  # 4 heads per core
DC = HPC * HD  # 256 head dims per core
P = 128
F32 = mybir.dt.float32
F32R = mybir.dt.float32r
AF = mybir.ActivationFunctionType


def _r(ap):
    return ap


def build_nc(S=2048, with_qk_bias=False):
    """Emit the per-core Bass program (SPMD; all cores run the same NEFF)."""
    SF = B * S  # flattened batch*seq
    NKC = HID // P  # 16 contraction chunks
    SB = 256  # s-block width in phase A
    NSB = SF // SB
    NSKT = S // P  # key tiles per batch
    NSQB = S // 512  # query blocks per batch
    NOB = HID // P  # output-proj row blocks
    NVC = SF // P  # v chunks

    nc = bacc.Bacc(None, target_bir_lowering=False, debug=False)

    xt_d = nc.dram_tensor("xt", [HID, SF], F32R, kind="ExternalInput")
    wq_d = nc.dram_tensor("wq", [HID, DC], F32R, kind="ExternalInput")
    wk_d = nc.dram_tensor("wk", [HID, DC], F32R, kind="ExternalInput")
    wv_d = nc.dram_tensor("wv", [HID, DC], F32R, kind="ExternalInput")
    wo_d = nc.dram_tensor("wo", [DC, HID], F32R, kind="ExternalInput")
    cs_d = nc.dram_tensor("cs", [2, P, S], F32, kind="ExternalInput")
    bvb_d = nc.dram_tensor("bvb", [P, DC], F32, kind="ExternalInput")
    mask_d = nc.dram_tensor("maskt", [P, NSKT, B], F32, kind="ExternalInput")
    von_d = nc.dram_tensor("vones", [P, NVC * HPC], F32R, kind="ExternalInput")
    if with_qk_bias:
        brq_d = nc.dram_tensor("brq", [2, P, S], F32, kind="ExternalInput")
        brk_d = nc.dram_tensor("brk", [2, P, S], F32, kind="ExternalInput")

    et_d = nc.dram_tensor("et_out", [HPC, B, S, S], F32R, kind="ExternalOutput")
    ot_d = nc.dram_tensor("ot_out", [HID, SF], F32, kind="ExternalOutput")
    rc_d = nc.dram_tensor("recip_out", [HPC, B, S], F32, kind="ExternalOutput")

    qt_d = nc.dram_tensor("qt_spill", [2, P, SF], F32R)  # internal scratch

    with TileContext(nc) as tc:
        with tc.tile_pool(name="pers", bufs=1) as pers:
            kT0 = pers.tile([P, SF], F32R)
            kT1 = pers.tile([P, SF], F32R)
            vx = pers.tile([P, NVC, HPC, HD + 1], F32R)
            maskT = pers.tile([P, NSKT, B], F32)
            bvb = pers.tile([P, DC], F32)

            nc.sync.dma_start(vx[:, :, :, HD : HD + 1], von_d.rearrange("p (a b) -> p a b", a=NVC)[:, :, :, None])
            nc.sync.dma_start(maskT[:], mask_d[:])
            nc.sync.dma_start(bvb[:], bvb_d[:])

            # ---------------- Phase A: QKV projections + RoPE ----------------
            with tc.tile_pool(name="wpool", bufs=1) as wpool, \
                 tc.tile_pool(name="xpool", bufs=3) as xpool, \
                 tc.tile_pool(name="cspool", bufs=2) as cspool, \
                 tc.tile_pool(name="stg", bufs=3) as stg, \
                 tc.tile_pool(name="psA", bufs=1, space="PSUM") as psA:
                wq = wpool.tile([P, NKC, DC], F32R)
                wk = wpool.tile([P, NKC, DC], F32R)
                wv = wpool.tile([P, NKC, DC], F32R)
                for kc in range(NKC):
                    nc.sync.dma_start(wq[:, kc], wq_d[kc * P : (kc + 1) * P, :])
                    nc.sync.dma_start(wk[:, kc], wk_d[kc * P : (kc + 1) * P, :])
                    nc.sync.dma_start(wv[:, kc], wv_d[kc * P : (kc + 1) * P, :])

                for sb in range(NSB):
                    c0 = sb * SB  # flat column offset
                    cin = c0 % S  # position within batch (cos/sin index)
                    xt = xpool.tile([P, NKC, SB], F32R)
                    for kc in range(NKC):
                        nc.sync.dma_start(
                            xt[:, kc], xt_d[kc * P : (kc + 1) * P, c0 : c0 + SB]
                        )
                    coss = cspool.tile([P, SB], F32)
                    sins = cspool.tile([P, SB], F32)
                    nc.sync.dma_start(coss[:], cs_d[0, :, cin : cin + SB])
                    nc.sync.dma_start(sins[:], cs_d[1, :, cin : cin + SB])

                    for mat in range(2):  # 0 = q, 1 = k
                        w = (wq, wk)[mat]
                        ps0 = psA.tile([P, SB], F32, tag="qk", bufs=4)
                        ps1 = psA.tile([P, SB], F32, tag="qk", bufs=4)
                        for t, pst in ((0, ps0), (1, ps1)):
                            for kc in range(NKC):
                                nc.tensor.matmul(
                                    pst[:],
                                    _r(w[:, kc, t * P : (t + 1) * P]),
                                    _r(xt[:, kc]),
                                    start=(kc == 0),
                                    stop=(kc == NKC - 1),
                                )
                        # RoPE: rows of tile0 pair with same rows of tile1.
                        if mat == 0:
                            o0 = stg.tile([P, SB], F32R, tag="o0")
                            o1 = stg.tile([P, SB], F32R, tag="o1")
                        else:
                            o0 = kT0[:, c0 : c0 + SB]
                            o1 = kT1[:, c0 : c0 + SB]
                        tmp = stg.tile([P, SB], F32, tag="tmp")
                        tmp2 = stg.tile([P, SB], F32, tag="tmp2")
                        nc.vector.tensor_mul(tmp, ps1[:], sins[:])
                        nc.vector.tensor_mul(o0, ps0[:], coss[:])
                        nc.vector.tensor_sub(o0, o0, tmp)
                        nc.vector.tensor_mul(tmp2, ps0[:], sins[:])
                        nc.vector.tensor_mul(o1, ps1[:], coss[:])
                        nc.vector.tensor_add(o1, o1, tmp2)
                        if with_qk_bias:
                            br = (brq_d, brk_d)[mat]
                            br0 = cspool.tile([P, SB], F32, tag="br0")
                            br1 = cspool.tile([P, SB], F32, tag="br1")
                            nc.sync.dma_start(br0[:], br[0, :, cin : cin + SB])
                            nc.sync.dma_start(br1[:], br[1, :, cin : cin + SB])
                            nc.vector.tensor_add(o0, o0, br0)
                            nc.vector.tensor_add(o1, o1, br1)
                        if mat == 0:
                            nc.sync.dma_start(qt_d[0, :, c0 : c0 + SB], o0)
                            nc.sync.dma_start(qt_d[1, :, c0 : c0 + SB], o1)

                    for ss in range(SB // P):  # v: natural layout
                        psv = psA.tile([P, DC], F32, tag="v", bufs=2)
                        for kc in range(NKC):
                            nc.tensor.matmul(
                                psv[:],
                                _r(xt[:, kc, ss * P : (ss + 1) * P]),
                                _r(wv[:, kc]),
                                start=(kc == 0),
                                stop=(kc == NKC - 1),
                            )
                        cidx = (c0 + ss * P) // P
                        for h in range(HPC):
                            nc.vector.tensor_add(
                                vx[:, cidx, h, 0:HD],
                                psv[:, h * HD : (h + 1) * HD],
                                bvb[:, h * HD : (h + 1) * HD],
                            )

            # ---------------- Phase B: attention + output projection ----------
            with tc.tile_pool(name="bpool", bufs=1) as bpool, \
                 tc.tile_pool(name="qrd", bufs=2) as qrd, \
                 tc.tile_pool(name="epool", bufs=4) as epool, \
                 tc.tile_pool(name="cpool", bufs=2) as cpool, \
                 tc.tile_pool(name="opool", bufs=3) as opool, \
                 tc.tile_pool(name="psS", bufs=2, space="PSUM") as psS, \
                 tc.tile_pool(name="psC", bufs=4, space="PSUM") as psC:
                wo = bpool.tile([P, 2, HID], F32R)
                for co in range(2):
                    nc.sync.dma_start(wo[:, co], wo_d[co * P : (co + 1) * P, :])

                for b in range(B):
                    for sqb in range(NSQB):
                        qc = b * S + sqb * 512
                        q0 = qrd.tile([P, 512], F32R, tag="q0")
                        q1 = qrd.tile([P, 512], F32R, tag="q1")
                        nc.sync.dma_start(q0[:], qt_d[0, :, qc : qc + 512])
                        nc.sync.dma_start(q1[:], qt_d[1, :, qc : qc + 512])
                        pc = [
                            psC.tile([HD + 1, 512], F32, tag="ctx", name=f"pc{h}")
                            for h in range(HPC)
                        ]
                        for skt in range(NSKT):
                            kc0 = b * S + skt * P
                            for pair in range(2):  # heads (0,1) then (2,3)
                                ps = psS.tile([P, 1024], F32, tag="sc")
                                for t, (kT, qn) in enumerate(((kT0, q0), (kT1, q1))):
                                    for hh in range(2):
                                        h = pair * 2 + hh
                                        nc.tensor.matmul(
                                            ps[:, hh * 512 : (hh + 1) * 512],
                                            _r(kT[32 * h : 32 * h + 32, kc0 : kc0 + P]),
                                            _r(qn[32 * h : 32 * h + 32, :]),
                                            start=(t == 0),
                                            stop=(t == 1),
                                            tile_position=(32 * h, 0),
                                        )
                                et = epool.tile([P, 1024], F32R)
                                nc.scalar.activation(
                                    et[:],
                                    ps[:],
                                    AF.Exp,
                                    bias=maskT[:, skt, b : b + 1],
                                    scale=1.0 / np.sqrt(HD),
                                )
                                for hh in range(2):
                                    h = pair * 2 + hh
                                    nc.sync.dma_start(
                                        et_d[
                                            h,
                                            b,
                                            skt * P : (skt + 1) * P,
                                            sqb * 512 : sqb * 512 + 512,
                                        ],
                                        et[:, hh * 512 : (hh + 1) * 512],
                                    )
                                    nc.tensor.matmul(
                                        pc[h][:],
                                        _r(vx[:, (b * S) // P + skt, h, :]),
                                        _r(et[:, hh * 512 : (hh + 1) * 512]),
                                        start=(skt == 0),
                                        stop=(skt == NSKT - 1),
                                    )
                        cn = cpool.tile([P, 2, 512], F32R)
                        for h in range(HPC):
                            rec = opool.tile([1, 512], F32, tag="rec")
                            nc.vector.reciprocal(rec[:], pc[h][HD : HD + 1, :])
                            nc.sync.dma_start(
                                rc_d[h, b : b + 1, sqb * 512 : sqb * 512 + 512],
                                rec[:],
                            )
                            pbs = opool.tile([HD, 512], F32, tag="pbs")
                            nc.gpsimd.partition_broadcast(pbs[:], rec[:])
                            nc.vector.tensor_mul(
                                cn[64 * (h % 2) : 64 * (h % 2) + 64, h // 2, :],
                                pc[h][0:HD, :],
                                pbs[:],
                            )
                        for ob in range(NOB):
                            po = psS.tile([P, 1024], F32, tag="sc", name=f"po{ob}")
                            for co in range(2):
                                nc.tensor.matmul(
                                    po[:, 0:512],
                                    _r(wo[:, co, ob * P : (ob + 1) * P]),
                                    _r(cn[:, co, :]),
                                    start=(co == 0),
                                    stop=(co == 1),
                                )
                            ot = opool.tile([P, 512], F32, tag="ot")
                            nc.vector.tensor_copy(ot[:], po[:, 0:512])
                            nc.sync.dma_start(
                                ot_d[ob * P : (ob + 1) * P, qc : qc + 512], ot[:]
                            )

    nc.compile()
    return nc


def prep_inputs(hidden_states, attention_mask, Wq, bq, Wk, bk, Wv, bv, Wo, bo, S):
    """Host-side shard + reorder. Returns (in_maps, with_qk_bias)."""
    SF = B * S
    X = np.ascontiguousarray(hidden_states.reshape(SF, HID).T)  # (HID, SF)

    idx = np.arange(P)
    ord0 = (idx // 32) * HD + (idx % 32)
    order = np.concatenate([ord0, ord0 + 32])  # (256,) local row reorder

    inv = (1.0 / 10000.0 ** (np.arange(0, HD, 2) / HD)).astype(np.float64)
    ang = np.outer(inv, np.arange(S))  # (32, S)
    cosb = np.cos(ang).astype(np.float32)
    sinb = np.sin(ang).astype(np.float32)
    cs = np.ascontiguousarray(
        np.stack([np.tile(cosb, (4, 1)), np.tile(sinb, (4, 1))])
    )  # (2, 128, S)

    m2 = np.asarray(attention_mask)[:, 0, 0, :]  # (B, S)
    maskT = np.ascontiguousarray(m2.reshape(B, S // P, P).transpose(2, 1, 0))

    with_qk_bias = bool(np.any(bq) or np.any(bk))
    in_maps = []
    for m in range(NCORES):
        sl = slice(m * DC, (m + 1) * DC)
        wqm, wkm, wvm = Wq[sl], Wk[sl], Wv[sl]
        im = {
            "xt": X,
            "vones": np.ones((P, (B * S // P) * HPC), np.float32),
            "wq": np.ascontiguousarray(wqm[order].T),
            "wk": np.ascontiguousarray(wkm[order].T),
            "wv": np.ascontiguousarray(wvm.T),
            "wo": np.ascontiguousarray(Wo[:, sl].T),
            "cs": cs,
            "bvb": np.ascontiguousarray(np.tile(bv[sl], (P, 1))),
            "maskt": maskT,
        }
        if with_qk_bias:
            for nm, bb in (("brq", bq[sl][order]), ("brk", bk[sl][order])):
                b0, b1 = bb[:P, None], bb[P:, None]
                im[nm] = np.ascontiguousarray(
                    np.stack([b0 * cs[0] - b1 * cs[1], b1 * cs[0] + b0 * cs[1]])
                )
            in_maps.append(im)
        else:
            in_maps.append(im)
    return in_maps, with_qk_bias


_NC_CACHE = {}


def _kernel_impl(inputs, S=2048, trace=False, tmpdir=None):
    in_maps, with_qk_bias = prep_inputs(S=S, **inputs)
    key = (S, with_qk_bias)
    if key not in _NC_CACHE:
        _NC_CACHE[key] = build_nc(S=S, with_qk_bias=with_qk_bias)
    nc = _NC_CACHE[key]
    res = run_bass_kernel_spmd(
        nc, in_maps, core_ids=list(range(NCORES)), trace=trace, tmpdir=tmpdir
    )

    bo = np.asarray(inputs["bo"])
    attn = np.empty((B, NH, S, S), np.float32)
    ot_sum = None
    for m in range(NCORES):
        r = res.results[m]
        et, rc, ot = r["et_out"], r["recip_out"], r["ot_out"]
        for h in range(HPC):
            for b in range(B):
                attn[b, m * HPC + h] = (et[h, b] * rc[h, b][None, :]).T
        ot_sum = ot if ot_sum is None else ot_sum + ot
    out = (ot_sum.T + bo[None, :]).reshape(B, S, HID).astype(np.float32)
    return (out, attn), res


def kernel(**inputs):
    (out, attn), _ = _kernel_impl(inputs)
    return out, attn


# revision 8
# speedup vs baseline: 1.0680x; 1.0680x over previous
"""Tensor-parallel multi-head attention (RoPE) for 8 Trainium2 NeuronCores.

Sharding: heads split 8 ways (4 heads/core). Each core computes its heads'
Q/K/V projections, RoPE, scores^T, exp (mask folded into the activation
bias), context (with an appended ones-column producing softmax row-sums for
free), and a partial output projection. Host assembles: normalizes +
transposes attn weights, sums partial output projections, adds biases.

Device-side layouts (per core):
  - q/k head dims are reordered as [head, half, j] so RoPE's rotate-half is
    a partition-aligned operation across all 4 heads at once.
  - scores are computed transposed (S^T[sk, sq]) so softmax's exp takes the
    attention mask as a per-partition bias, E^T feeds the context matmul
    directly as lhsT, and no on-device transposes are needed anywhere.
"""

import os
import sys

import numpy as np

for _p in ("/opt/trn_rl_repo", "/root/.axon_site/_ro/trn_rl_repo"):
    if os.path.isdir(_p) and _p not in sys.path:
        sys.path.insert(0, _p)

import concourse.bass as bass  # noqa: E402
import concourse.mybir as mybir  # noqa: E402
from concourse import bacc  # noqa: E402
from concourse.bass_utils import run_bass_kernel_spmd  # noqa: E402
from concourse.tile import TileContext  # noqa: E402

B, HID, NH, HD = 2, 2048, 32, 64
NCORES = 8
HPC = NH // NCORES  # 4 heads per core
DC = HPC * HD  # 256 head dims per core
P = 128
F32 = mybir.dt.float32
F32R = mybir.dt.float32r
AF = mybir.ActivationFunctionType


def _r(ap):
    return ap


def build_nc(S=2048, with_qk_bias=False):
    """Emit the per-core Bass program (SPMD; all cores run the same NEFF)."""
    SF = B * S  # flattened batch*seq
    NKC = HID // P  # 16 contraction chunks
    SB = 256  # s-block width in phase A
    NSB = SF // SB
    NSKT = S // P  # key tiles per batch
    NSQB = S // 512  # query blocks per batch
    NOB = HID // P  # output-proj row blocks
    NVC = SF // P  # v chunks

    nc = bacc.Bacc(None, target_bir_lowering=False, debug=False)

    xt_d = nc.dram_tensor("xt", [HID, SF], F32R, kind="ExternalInput")
    wq_d = nc.dram_tensor("wq", [HID, DC], F32R, kind="ExternalInput")
    wk_d = nc.dram_tensor("wk", [HID, DC], F32R, kind="ExternalInput")
    wv_d = nc.dram_tensor("wv", [HID, DC], F32R, kind="ExternalInput")
    wo_d = nc.dram_tensor("wo", [DC, HID], F32R, kind="ExternalInput")
    cs_d = nc.dram_tensor("cs", [2, P, S], F32, kind="ExternalInput")
    bvb_d = nc.dram_tensor("bvb", [P, DC], F32, kind="ExternalInput")
    mask_d = nc.dram_tensor("maskt", [P, NSKT, B], F32, kind="ExternalInput")
    von_d = nc.dram_tensor("vones", [P, NVC * HPC], F32R, kind="ExternalInput")
    if with_qk_bias:
        brq_d = nc.dram_tensor("brq", [2, P, S], F32, kind="ExternalInput")
        brk_d = nc.dram_tensor("brk", [2, P, S], F32, kind="ExternalInput")

    et_d = nc.dram_tensor("et_out", [HPC, B, S, S], F32R, kind="ExternalOutput")
    ot_d = nc.dram_tensor("ot_out", [HID, SF], F32, kind="ExternalOutput")
    rc_d = nc.dram_tensor("recip_out", [HPC, B, S], F32, kind="ExternalOutput")

    qt_d = nc.dram_tensor("qt_spill", [2, P, SF], F32R)  # internal scratch

    with TileContext(nc) as tc:
        with tc.tile_pool(name="pers", bufs=1) as pers:
            kT0 = pers.tile([P, SF], F32R)
            kT1 = pers.tile([P, SF], F32R)
            vx = pers.tile([P, NVC, HPC, HD + 1], F32R)
            maskT = pers.tile([P, NSKT, B], F32)
            bvb = pers.tile([P, DC], F32)

            nc.sync.dma_start(vx[:, :, :, HD : HD + 1], von_d.rearrange("p (a b) -> p a b", a=NVC)[:, :, :, None])
            nc.sync.dma_start(maskT[:], mask_d[:])
            nc.sync.dma_start(bvb[:], bvb_d[:])

            # ---------------- Phase A: QKV projections + RoPE ----------------
            with tc.tile_pool(name="wpool", bufs=1) as wpool, \
                 tc.tile_pool(name="xpool", bufs=3) as xpool, \
                 tc.tile_pool(name="cspool", bufs=2) as cspool, \
                 tc.tile_pool(name="stg", bufs=3) as stg, \
                 tc.tile_pool(name="psA", bufs=1, space="PSUM") as psA:
                wq = wpool.tile([P, NKC, DC], F32R)
                wk = wpool.tile([P, NKC, DC], F32R)
                wv = wpool.tile([P, NKC, DC], F32R)
                for kc in range(NKC):
                    nc.sync.dma_start(wq[:, kc], wq_d[kc * P : (kc + 1) * P, :])
                    nc.sync.dma_start(wk[:, kc], wk_d[kc * P : (kc + 1) * P, :])
                    nc.sync.dma_start(wv[:, kc], wv_d[kc * P : (kc + 1) * P, :])

                for sb in range(NSB):
                    c0 = sb * SB  # flat column offset
                    cin = c0 % S  # position within batch (cos/sin index)
                    xt = xpool.tile([P, NKC, SB], F32R)
                    for kc in range(NKC):
                        nc.sync.dma_start(
                            xt[:, kc], xt_d[kc * P : (kc + 1) * P, c0 : c0 + SB]
                        )
                    coss = cspool.tile([P, SB], F32)
                    sins = cspool.tile([P, SB], F32)
                    nc.sync.dma_start(coss[:], cs_d[0, :, cin : cin + SB])
                    nc.sync.dma_start(sins[:], cs_d[1, :, cin : cin + SB])

                    for mat in range(2):  # 0 = q, 1 = k
                        w = (wq, wk)[mat]
                        ps0 = psA.tile([P, SB], F32, tag="qk", bufs=4)
                        ps1 = psA.tile([P, SB], F32, tag="qk", bufs=4)
                        for t, pst in ((0, ps0), (1, ps1)):
                            for kc in range(NKC):
                                nc.tensor.matmul(
                                    pst[:],
                                    _r(w[:, kc, t * P : (t + 1) * P]),
                                    _r(xt[:, kc]),
                                    start=(kc == 0),
                                    stop=(kc == NKC - 1),
                                )
                        # RoPE: rows of tile0 pair with same rows of tile1.
                        if mat == 0:
                            o0 = stg.tile([P, SB], F32R, tag="o0")
                            o1 = stg.tile([P, SB], F32R, tag="o1")
                        else:
                            o0 = kT0[:, c0 : c0 + SB]
                            o1 = kT1[:, c0 : c0 + SB]
                        tmp = stg.tile([P, SB], F32, tag="tmp")
                        tmp2 = stg.tile([P, SB], F32, tag="tmp2")
                        nc.vector.tensor_mul(tmp, ps1[:], sins[:])
                        nc.vector.tensor_mul(o0, ps0[:], coss[:])
                        nc.vector.tensor_sub(o0, o0, tmp)
                        nc.vector.tensor_mul(tmp2, ps0[:], sins[:])
                        nc.vector.tensor_mul(o1, ps1[:], coss[:])
                        nc.vector.tensor_add(o1, o1, tmp2)
                        if with_qk_bias:
                            br = (brq_d, brk_d)[mat]
                            br0 = cspool.tile([P, SB], F32, tag="br0")
                            br1 = cspool.tile([P, SB], F32, tag="br1")
                            nc.sync.dma_start(br0[:], br[0, :, cin : cin + SB])
                            nc.sync.dma_start(br1[:], br[1, :, cin : cin + SB])
                            nc.vector.tensor_add(o0, o0, br0)
                            nc.vector.tensor_add(o1, o1, br1)
                        if mat == 0:
                            nc.sync.dma_start(qt_d[0, :, c0 : c0 + SB], o0)
                            nc.sync.dma_start(qt_d[1, :, c0 : c0 + SB], o1)

                    for ss in range(SB // P):  # v: natural layout
                        psv = psA.tile([P, DC], F32, tag="v", bufs=2)
                        for kc in range(NKC):
                            nc.tensor.matmul(
                                psv[:],
                                _r(xt[:, kc, ss * P : (ss + 1) * P]),
                                _r(wv[:, kc]),
                                start=(kc == 0),
                                stop=(kc == NKC - 1),
                            )
                        cidx = (c0 + ss * P) // P
                        for h in range(HPC):
                            nc.vector.tensor_add(
                                vx[:, cidx, h, 0:HD],
                                psv[:, h * HD : (h + 1) * HD],
                                bvb[:, h * HD : (h + 1) * HD],
                            )

            # ---------------- Phase B: attention + output projection ----------
            # Software-pipelined: block i's normalization + output projection
            # run under block i+1's score matmuls so the PE never idles long
            # enough for the HAM clock gate to re-throttle.
            with tc.tile_pool(name="bpool", bufs=1) as bpool, \
                 tc.tile_pool(name="qrd", bufs=2) as qrd, \
                 tc.tile_pool(name="epool", bufs=6) as epool, \
                 tc.tile_pool(name="cpool", bufs=2) as cpool, \
                 tc.tile_pool(name="opool", bufs=3) as opool, \
                 tc.tile_pool(name="psS", bufs=2, space="PSUM") as psS, \
                 tc.tile_pool(name="psC", bufs=4, space="PSUM") as psC:
                wo = bpool.tile([P, 2, HID], F32R)
                for co in range(2):
                    nc.sync.dma_start(wo[:, co], wo_d[co * P : (co + 1) * P, :])

                def scores_skt(b, sqb, skt, q0, q1, ets):
                    """Two full-row waves -> 4 heads' scores^T, exp, DMA out."""
                    kc0 = b * S + skt * P
                    psab = [
                        psS.tile([P, 1024], F32, tag="sc", name=f"ps{skt}_{i}")
                        for i in range(2)
                    ]
                    for t, (kT, qn) in enumerate(((kT0, q0), (kT1, q1))):
                        for h in range(HPC):
                            nc.tensor.matmul(
                                psab[h // 2][:, (h % 2) * 512 : (h % 2) * 512 + 512],
                                kT[32 * h : 32 * h + 32, kc0 : kc0 + P],
                                qn[32 * h : 32 * h + 32, :],
                                start=(t == 0),
                                stop=(t == 1),
                                tile_position=(32 * h, 0),
                            )
                    pair_ets = []
                    for i in range(2):
                        et = epool.tile([P, 1024], F32R, tag="et", name=f"et{skt}_{i}")
                        nc.scalar.activation(
                            et[:],
                            psab[i][:],
                            AF.Exp,
                            bias=maskT[:, skt, b : b + 1],
                            scale=1.0 / np.sqrt(HD),
                        )
                        for hh in range(2):
                            nc.sync.dma_start(
                                et_d[
                                    i * 2 + hh,
                                    b,
                                    skt * P : (skt + 1) * P,
                                    sqb * 512 : sqb * 512 + 512,
                                ],
                                et[:, hh * 512 : (hh + 1) * 512],
                            )
                        pair_ets.append(et)
                    ets.append((skt, pair_ets))

                def ctx_mms(b, skt, pc, pair_ets):
                    for h in range(HPC):
                        nc.tensor.matmul(
                            pc[h][:],
                            vx[:, (b * S) // P + skt, h, :],
                            pair_ets[h // 2][:, (h % 2) * 512 : (h % 2) * 512 + 512],
                            start=(skt == 0),
                            stop=(skt == NSKT - 1),
                        )

                def norm_oproj(b, sqb, pc):
                    qc = b * S + sqb * 512
                    cn = cpool.tile([P, 2, 512], F32R)
                    for h in range(HPC):
                        rinv = opool.tile([1, 512], F32, tag=f"rinv{h % 2}")
                        nc.vector.reciprocal(rinv[:], pc[h][HD : HD + 1, :])
                        nc.sync.dma_start(
                            rc_d[h, b : b + 1, sqb * 512 : sqb * 512 + 512], rinv[:]
                        )
                        pbs = opool.tile([HD, 512], F32, tag=f"pbs{h % 2}")
                        nc.gpsimd.partition_broadcast(pbs[:], rinv[:])
                        nc.vector.tensor_mul(
                            cn[64 * (h % 2) : 64 * (h % 2) + 64, h // 2, :],
                            pc[h][0:HD, :],
                            pbs[:],
                        )
                    for ob in range(NOB):
                        po = psS.tile([P, 1024], F32, tag="sc", name=f"po{ob}")
                        for co in range(2):
                            nc.tensor.matmul(
                                po[:, 0:512],
                                wo[:, co, ob * P : (ob + 1) * P],
                                cn[:, co, :],
                                start=(co == 0),
                                stop=(co == 1),
                            )
                        ot = opool.tile([P, 512], F32, tag="ot")
                        nc.vector.tensor_copy(ot[:], po[:, 0:512])
                        nc.sync.dma_start(
                            ot_d[ob * P : (ob + 1) * P, qc : qc + 512], ot[:]
                        )

                prev = None  # (b, sqb, pc) awaiting normalization + oproj
                for b in range(B):
                    for sqb in range(NSQB):
                        qc = b * S + sqb * 512
                        q0 = qrd.tile([P, 512], F32R, tag="q0")
                        q1 = qrd.tile([P, 512], F32R, tag="q1")
                        nc.sync.dma_start(q0[:], qt_d[0, :, qc : qc + 512])
                        nc.sync.dma_start(q1[:], qt_d[1, :, qc : qc + 512])
                        pc = [
                            psC.tile([HD + 1, 512], F32, tag="ctx", name=f"pc{h}")
                            for h in range(HPC)
                        ]
                        ets = []
                        scores_skt(b, sqb, 0, q0, q1, ets)
                        scores_skt(b, sqb, 1, q0, q1, ets)
                        if prev is not None:
                            norm_oproj(*prev)
                        for skt, pair_ets in ets:
                            ctx_mms(b, skt, pc, pair_ets)
                        ets.clear()
                        for skt in range(2, NSKT):
                            scores_skt(b, sqb, skt, q0, q1, ets)
                            ctx_mms(b, skt, pc, ets.pop()[1])
                        prev = (b, sqb, pc)
                norm_oproj(*prev)

    nc.compile()
    return nc


def prep_inputs(hidden_states, attention_mask, Wq, bq, Wk, bk, Wv, bv, Wo, bo, S):
    """Host-side shard + reorder. Returns (in_maps, with_qk_bias)."""
    SF = B * S
    X = np.ascontiguousarray(hidden_states.reshape(SF, HID).T)  # (HID, SF)

    idx = np.arange(P)
    ord0 = (idx // 32) * HD + (idx % 32)
    order = np.concatenate([ord0, ord0 + 32])  # (256,) local row reorder

    inv = (1.0 / 10000.0 ** (np.arange(0, HD, 2) / HD)).astype(np.float64)
    ang = np.outer(inv, np.arange(S))  # (32, S)
    cosb = np.cos(ang).astype(np.float32)
    sinb = np.sin(ang).astype(np.float32)
    cs = np.ascontiguousarray(
        np.stack([np.tile(cosb, (4, 1)), np.tile(sinb, (4, 1))])
    )  # (2, 128, S)

    m2 = np.asarray(attention_mask)[:, 0, 0, :]  # (B, S)
    maskT = np.ascontiguousarray(m2.reshape(B, S // P, P).transpose(2, 1, 0))

    with_qk_bias = bool(np.any(bq) or np.any(bk))
    in_maps = []
    for m in range(NCORES):
        sl = slice(m * DC, (m + 1) * DC)
        wqm, wkm, wvm = Wq[sl], Wk[sl], Wv[sl]
        im = {
            "xt": X,
            "vones": np.ones((P, (B * S // P) * HPC), np.float32),
            "wq": np.ascontiguousarray(wqm[order].T),
            "wk": np.ascontiguousarray(wkm[order].T),
            "wv": np.ascontiguousarray(wvm.T),
            "wo": np.ascontiguousarray(Wo[:, sl].T),
            "cs": cs,
            "bvb": np.ascontiguousarray(np.tile(bv[sl], (P, 1))),
            "maskt": maskT,
        }
        if with_qk_bias:
            for nm, bb in (("brq", bq[sl][order]), ("brk", bk[sl][order])):
                b0, b1 = bb[:P, None], bb[P:, None]
                im[nm] = np.ascontiguousarray(
                    np.stack([b0 * cs[0] - b1 * cs[1], b1 * cs[0] + b0 * cs[1]])
                )
            in_maps.append(im)
        else:
            in_maps.append(im)
    return in_maps, with_qk_bias


_NC_CACHE = {}


def _kernel_impl(inputs, S=2048, trace=False, tmpdir=None):
    in_maps, with_qk_bias = prep_inputs(S=S, **inputs)
    key = (S, with_qk_bias)
    if key not in _NC_CACHE:
        _NC_CACHE[key] = build_nc(S=S, with_qk_bias=with_qk_bias)
    nc = _NC_CACHE[key]
    res = run_bass_kernel_spmd(
        nc, in_maps, core_ids=list(range(NCORES)), trace=trace, tmpdir=tmpdir
    )

    bo = np.asarray(inputs["bo"])
    attn = np.empty((B, NH, S, S), np.float32)
    ot_sum = None
    for m in range(NCORES):
        r = res.results[m]
        et, rc, ot = r["et_out"], r["recip_out"], r["ot_out"]
        for h in range(HPC):
            for b in range(B):
                attn[b, m * HPC + h] = (et[h, b] * rc[h, b][None, :]).T
        ot_sum = ot if ot_sum is None else ot_sum + ot
    out = (ot_sum.T + bo[None, :]).reshape(B, S, HID).astype(np.float32)
    return (out, attn), res


def kernel(**inputs):
    (out, attn), _ = _kernel_impl(inputs)
    return out, attn
